# revision 7
# baseline (speedup 1.0000x reference)
"""Quantized int8 conv2d (brevitas-style) on 8 TRN2 NeuronCores.

Data-parallel over batch (1 image / core). Per-tensor symmetric int8
quantization: local abs-max -> AllReduce(max) -> quantize -> 3x3 conv
(stride 1, pad 1) as PE-tiled 32x32 matmuls -> dequant + bias.

Key tricks:
- x is cached in SBUF as fp16 during the abs-max pass (single DRAM read).
- round(v) is computed as fp16(v + 1536) (fp16 RNE at the [1024,2048)
  binade has ulp=1 -> exact round-half-even, matching jnp.round). The
  +1536 offset is NOT subtracted elementwise; instead it rides through
  the conv matmuls and is cancelled exactly by correction matmuls on the
  4 otherwise-idle PE tile positions, using {0,1536} column patterns that
  replicate the zero-padding tap structure.
- All matmul operands are fp16 (integer values up to 2048 are exact).
"""

import sys

if "/opt/trn_rl_repo" not in sys.path:
    sys.path.insert(0, "/opt/trn_rl_repo")

import numpy as np

import concourse.bass as bass
import concourse.bacc as bacc
import concourse.mybir as mybir
from concourse import tile
from concourse.bass_utils import run_bass_kernel_spmd

N_CORES = 8
C = 32  # in channels
O = 32  # out channels
H = 512
W = 512
NQ = H // 4  # 128 four-row blocks
F32 = mybir.dt.float32
F16 = mybir.dt.float16

MAXV = 127.0
RND = 1536.0  # rounding anchor: [1536-127, 1536+127] subset of [1024, 2048)

# per-kw output/rhs column windows: (out_start, rhs_start, n)
KW_COLS = {0: (1, 0, 511), 1: (0, 0, 512), 2: (0, 1, 511)}


def build_nc(h=H):
    nc = bacc.Bacc(None, target_bir_lowering=False, debug=False)
    NQ = h // 4

    x_ext = nc.declare_dram_parameter("x", [C, h, W], F32, isOutput=False)
    w_ext = nc.declare_dram_parameter("weight", [O, C, 3, 3], F32, isOutput=False)
    b_ext = nc.declare_dram_parameter("bias", [O], F32, isOutput=False)
    out_ext = nc.declare_dram_parameter("out", [O, h, W], F32, isOutput=True)

    # collective bounce buffers (collectives can't touch I/O tensors)
    cc_in = nc.dram_tensor("cc_in", [128], F32)
    cc_out = nc.dram_tensor("cc_out", [128], F32, addr_space="Shared")

    with tile.TileContext(nc) as tc:
        with (
            tc.tile_pool(name="persist", bufs=1) as persist,
            tc.tile_pool(name="stage", bufs=4) as stage,
            tc.tile_pool(name="qx", bufs=6) as qxp,
            tc.tile_pool(name="outp", bufs=4) as outp,
            tc.tile_pool(name="spp", bufs=3) as spp,
            tc.tile_pool(name="snp", bufs=3) as snp,
            tc.tile_pool(name="psum", bufs=4, space="PSUM") as psump,
            tc.tile_pool(name="bpsum", bufs=2, space="PSUM") as bpsump,
        ):
            # ---------------- persistent SBUF tensors ----------------
            x4 = persist.tile([128, NQ * W], F16)      # fp16 cache of x
            maxes = persist.tile([128, NQ], F32)       # per-q absmax columns
            wsb = persist.tile([128, 288], F32)        # w as [i, (kh kw o)] x4 groups
            qw = persist.tile([128, 288], F16)         # quantized weights
            tq = persist.tile([128, 288], F16)         # temp (qw + 1536)
            cw = persist.tile([128, 288], F16)         # correction lhsT (3 variants)
            qwc = persist.tile([128, 384], F16)        # main lhsT: (c,kw) blocks, rows (hm,i)
            cw4 = persist.tile([96, 96], F16)          # corr lhsT: rows (kw,i), cols v*32+o
            bw = persist.tile([96, 64], F16)           # boundary lhsT: rows (kw,i), cols e*32+o
            cb4 = persist.tile([96, W], F16)           # corr rhs patterns, rows (kw,i)
            ones_l = persist.tile([1, 128], F32)       # bcast matmul lhsT
            bias_sb = persist.tile([128, 1], F32)      # bias per partition (c*32+o)
            gmax = persist.tile([128, 1], F32)         # local per-partition absmax
            gmax2 = persist.tile([128, 1], F32)        # global per-partition absmax
            sg = persist.tile([1, 1], F32)             # global scale sx
            sw = persist.tile([1, 1], F32)             # weight scale
            inv = persist.tile([1, 1], F32)
            invw = persist.tile([1, 1], F32)
            cwi = persist.tile([1, 1], F32)            # 127/sw
            cqi = persist.tile([1, 1], F32)            # 127/sx
            dqi = persist.tile([1, 1], F32)            # sx*sw/127^2
            bc_in = persist.tile([1, 4], F32)          # bcast payload
            bvec = persist.tile([128, 4], F32)         # broadcast scalars
            s01 = persist.tile([128, 96], F16)         # qw kh0+kh1
            s12 = persist.tile([128, 96], F16)         # qw kh1+kh2
            sall = persist.tile([128, 96], F16)        # all kh

            # ---------------- weight path (local, no collective) -----
            # wsb[32g+i, kh*96+kw*32+o] = w[o,i,kh,kw], replicated g=0..3
            wv = w_ext[:, :, :, :].rearrange("o i kh kw -> i kh kw o")
            for g in range(4):
                nc.sync.dma_start(out=wsb[32 * g : 32 * g + 32, :], in_=wv)
            for cix in range(4):
                nc.sync.dma_start(
                    out=bias_sb[32 * cix : 32 * cix + 32, :], in_=b_ext[:, None]
                )
            nc.gpsimd.memset(ones_l[:, :], 1.0)
            nc.gpsimd.memset(qwc[:, :], 0.0)
            nc.gpsimd.memset(cb4[:, :], RND)
            nc.gpsimd.memset(cb4[0:32, 0:1], 0.0)        # kw=0 pattern, w=0
            nc.gpsimd.memset(cb4[64:96, W - 1 : W], 0.0)  # kw=2 pattern, w=511

            # sw = max |w|
            wred = persist.tile([128, 1], F32)
            nc.vector.tensor_reduce(
                out=wred[:, :],
                in_=wsb[:, :],
                axis=mybir.AxisListType.X,
                op=mybir.AluOpType.max,
                apply_absolute_value=True,
            )
            nc.gpsimd.tensor_reduce(
                out=sw[:, :],
                in_=wred[:, :],
                axis=mybir.AxisListType.C,
                op=mybir.AluOpType.max,
            )
            nc.vector.reciprocal(invw[:, :], sw[:, :])
            nc.vector.tensor_scalar_mul(cwi[:, :], invw[:, :], MAXV)

            # broadcast 127/sw to all partitions: psum = ones_l.T @ cwi
            bps = bpsump.tile([128, 4], F32)
            nc.tensor.matmul(bps[:, 0:1], ones_l[:, :], cwi[:, :])
            cw_ap = persist.tile([128, 1], F32)
            nc.vector.tensor_copy(cw_ap[:, :], bps[:, 0:1])

            # qw = round(w * 127/sw) via fp16 +1536 trick
            nc.scalar.activation(
                out=tq[:, :], in_=wsb[:, :],
                func=mybir.ActivationFunctionType.Copy,
                scale=cw_ap[:, 0:1], bias=RND,
            )
            with nc.allow_low_precision("int8 values are exact in fp16"):
                nc.vector.tensor_scalar_add(qw[:, :], tq[:, :], -RND)
                # correction lhsT: variants full / no-kh0 / no-kh2, negated
                nc.vector.tensor_add(s01[:, :], qw[:, 0:96], qw[:, 96:192])
                nc.vector.tensor_add(s12[:, :], qw[:, 96:192], qw[:, 192:288])
                nc.vector.tensor_add(sall[:, :], s01[:, :], qw[:, 192:288])
                nc.vector.tensor_scalar_mul(cw[:, 0:96], sall[:, :], -1.0)
                nc.vector.tensor_scalar_mul(cw[:, 96:192], s12[:, :], -1.0)
                nc.vector.tensor_scalar_mul(cw[:, 192:288], s01[:, :], -1.0)
                # main lhsT blocks: qwc[32*hm+i, (c*3+kw)*32+o] = qw[o,i,hm-c+1,kw]
                for cix in range(4):
                    for kw in range(3):
                        for kh in range(3):
                            hm = cix + kh - 1
                            if not (0 <= hm <= 3):
                                continue
                            nc.vector.tensor_copy(
                                qwc[32 * hm : 32 * hm + 32,
                                    (cix * 3 + kw) * 32 : (cix * 3 + kw) * 32 + 32],
                                qw[0:32, kh * 96 + kw * 32 : kh * 96 + kw * 32 + 32],
                            )
                # corr lhsT: cw4[32*kw+i, v*32+o] = cw[i, v*96+kw*32+o]
                for v in range(3):
                    for kw in range(3):
                        nc.vector.tensor_copy(
                            cw4[32 * kw : 32 * kw + 32, v * 32 : v * 32 + 32],
                            cw[0:32, v * 96 + kw * 32 : v * 96 + kw * 32 + 32],
                        )
                # boundary lhsT: bw[32*kw+i, e*32+o] = qw[o,i,(0 if e==0 else 2),kw]
                for e, kh in ((0, 0), (1, 2)):
                    for kw in range(3):
                        nc.vector.tensor_copy(
                            bw[32 * kw : 32 * kw + 32, e * 32 : e * 32 + 32],
                            qw[0:32, kh * 96 + kw * 32 : kh * 96 + kw * 32 + 32],
                        )

            # ---------------- pass 1: stream x, absmax + fp16 cache ---
            for q in range(NQ):
                stg = stage.tile([128, W], F32)
                xv = x_ext[:, 4 * q : 4 * q + 4, :].rearrange("i hm w -> hm i w")
                nc.sync.dma_start(out=stg[:, :], in_=xv)
                nc.scalar.activation(
                    out=x4[:, q * W : (q + 1) * W], in_=stg[:, :],
                    func=mybir.ActivationFunctionType.Copy,
                )
                nc.vector.tensor_reduce(
                    out=maxes[:, q : q + 1], in_=stg[:, :],
                    axis=mybir.AxisListType.X,
                    op=mybir.AluOpType.max,
                    apply_absolute_value=True,
                )

            nc.vector.tensor_reduce(
                out=gmax[:, :], in_=maxes[:, :],
                axis=mybir.AxisListType.X,
                op=mybir.AluOpType.max,
            )

            # ---------------- all-reduce(max) across 8 cores ----------
            nc.sync.dma_start(out=cc_in[:, None], in_=gmax[:, :])
            nc.gpsimd.collective_compute(
                "AllReduce",
                mybir.AluOpType.max,
                replica_groups=[list(range(N_CORES))],
                ins=[cc_in[:].opt()],
                outs=[cc_out[:].opt()],
            )
            nc.sync.dma_start(out=gmax2[:, :], in_=cc_out[:, None])
            nc.gpsimd.tensor_reduce(
                out=sg[:, :], in_=gmax2[:, :],
                axis=mybir.AxisListType.C,
                op=mybir.AluOpType.max,
            )

            # scalars: cq = 127/sx, dq = sx*sw/127^2
            nc.vector.reciprocal(inv[:, :], sg[:, :])
            nc.vector.tensor_scalar_mul(cqi[:, :], inv[:, :], MAXV)
            nc.vector.tensor_mul(dqi[:, :], sg[:, :], sw[:, :])
            nc.vector.tensor_scalar_mul(dqi[:, :], dqi[:, :], 1.0 / (MAXV * MAXV))
            nc.vector.tensor_copy(bc_in[:, 0:1], cqi[:, :])
            nc.vector.tensor_copy(bc_in[:, 1:2], dqi[:, :])
            bps2 = bpsump.tile([128, 4], F32)
            nc.tensor.matmul(bps2[:, 0:2], ones_l[:, :], bc_in[:, 0:2])
            nc.vector.tensor_copy(bvec[:, 0:2], bps2[:, 0:2])
            cq_ap = bvec[:, 0:1]
            dq_ap = bvec[:, 1:2]

            # ---------------- pass 2: quantize + conv + dequant -------
            qx_tiles = {}

            def quantize_block(j):
                t = qxp.tile([128, W], F16)
                nc.scalar.activation(
                    out=t[:, :], in_=x4[:, j * W : (j + 1) * W],
                    func=mybir.ActivationFunctionType.Copy,
                    scale=cq_ap, bias=RND,
                )
                qx_tiles[j] = t

            quantize_block(0)
            quantize_block(1)

            for q in range(NQ):
                if q + 2 <= NQ - 1:
                    quantize_block(q + 2)

                # boundary staging: 3 kw-shifted copies of the edge rows
                sp = sn = None
                if q > 0:
                    sp = spp.tile([96, W], F16)
                    prev = qx_tiles[q - 1]
                    nc.sync.dma_start(out=sp[0:32, 1:W], in_=prev[96:128, 0 : W - 1])
                    nc.gpsimd.memset(sp[0:32, 0:1], 0.0)
                    nc.sync.dma_start(out=sp[32:64, 0:W], in_=prev[96:128, 0:W])
                    nc.sync.dma_start(out=sp[64:96, 0 : W - 1], in_=prev[96:128, 1:W])
                    nc.gpsimd.memset(sp[64:96, W - 1 : W], 0.0)
                if q < NQ - 1:
                    sn = snp.tile([96, W], F16)
                    nxt = qx_tiles[q + 1]
                    nc.sync.dma_start(out=sn[0:32, 1:W], in_=nxt[0:32, 0 : W - 1])
                    nc.gpsimd.memset(sn[0:32, 0:1], 0.0)
                    nc.sync.dma_start(out=sn[32:64, 0:W], in_=nxt[0:32, 0:W])
                    nc.sync.dma_start(out=sn[64:96, 0 : W - 1], in_=nxt[0:32, 1:W])
                    nc.gpsimd.memset(sn[64:96, W - 1 : W], 0.0)

                ps = psump.tile([128, W], F32)
                cur = qx_tiles[q]
                for cix in range(4):
                    mms = []
                    # main taps: K=128 over (hm,i), zero rows where invalid
                    for kw in (1, 0, 2):
                        oc0, rc0, nn = KW_COLS[kw]
                        mms.append(
                            (
                                qwc[0:128,
                                    (cix * 3 + kw) * 32 : (cix * 3 + kw) * 32 + 32],
                                cur[0:128, rc0 : rc0 + nn],
                                ps[32 * cix : 32 * cix + 32, oc0 : oc0 + nn],
                            )
                        )
                    # +1536 cancellation
                    v = 1 if (q == 0 and cix == 0) else (
                        2 if (q == NQ - 1 and cix == 3) else 0)
                    mms.append(
                        (
                            cw4[0:96, v * 32 : v * 32 + 32],
                            cb4[0:96, 0:W],
                            ps[32 * cix : 32 * cix + 32, 0:W],
                        )
                    )
                    # boundary rows (kh=0 from block q-1 / kh=2 from q+1)
                    if cix == 0 and sp is not None:
                        mms.append(
                            (bw[0:96, 0:32], sp[0:96, 0:W],
                             ps[0:32, 0:W])
                        )
                    if cix == 3 and sn is not None:
                        mms.append(
                            (bw[0:96, 32:64], sn[0:96, 0:W],
                             ps[96:128, 0:W])
                        )
                    for mi, (lhsT, rhs, outap) in enumerate(mms):
                        nc.tensor.matmul(
                            outap, lhsT, rhs,
                            start=(mi == 0),
                            stop=(mi == len(mms) - 1),
                            tile_position=(0, 32 * cix),
                        )

                ot = outp.tile([128, W], F32)
                nc.vector.tensor_scalar(
                    out=ot[:, :], in0=ps[:, :],
                    scalar1=dq_ap, scalar2=bias_sb[:, 0:1],
                    op0=mybir.AluOpType.mult,
                    op1=mybir.AluOpType.add,
                )
                ov = out_ext[:, 4 * q : 4 * q + 4, :].rearrange("o hm w -> hm o w")
                nc.sync.dma_start(out=ov, in_=ot[:, :])

    nc.finalize()
    return nc


_NC_CACHE = {}


def kernel(x, weight, bias):
    x = np.ascontiguousarray(x, dtype=np.float32)
    weight = np.ascontiguousarray(weight, dtype=np.float32)
    bias = np.ascontiguousarray(bias, dtype=np.float32)

    if "nc" not in _NC_CACHE:
        _NC_CACHE["nc"] = build_nc()
    nc = _NC_CACHE["nc"]

    in_maps = [
        {"x": x[i], "weight": weight, "bias": bias} for i in range(N_CORES)
    ]
    res = run_bass_kernel_spmd(nc, in_maps, core_ids=list(range(N_CORES)))
    outs = [res.results[i]["out"] for i in range(N_CORES)]
    return np.stack(outs, axis=0)


if __name__ == "__main__":
    xs = np.random.randn(8, 32, 64, 64).astype(np.float32)
    print("smoke build only")
    build_nc()
    print("build ok")
